# revision 15
# baseline (speedup 1.0000x reference)
"""3D LUT trilinear interpolation (color grading) on 8 Trainium2 NeuronCores.

Strategy (per core, data-parallel over batch):
  - Pack the 33^3 LUT into per-cell corner tables (32^3 cells x 8 corners x 3
    channels), fp16, laid out so one GPSIMD ap_gather index per pixel fetches
    all 24 values across 16 partition rows (d=2).
  - Per 8192-pixel tile: compute cell indices pixel-parallel, PE-transpose
    them into the gather's wrapped [16 x 64] index layout; compute fractions
    in stream layout, PE 0/1-matmuls replicate them into all 16 slot rows
    with +-0.5 bias folded in; 2 DVE ops build the 16 signed trilinear
    weights; multiply with gathered corners; 2 PE matmuls per 512-chunk
    contract the 8 corners per channel into PSUM.
"""

import numpy as np

LUT_DIM = 33
NCELL = 32 * 32 * 32      # 32768
N = 1024                  # stream pixels per 16-partition group per tile
PIX_TILE = 8 * N          # 8192 pixels per tile
NPIX = 3 * 1080 * 1920 // 3  # per-core pixels = 2073600
NTILES_FULL = -(-NPIX // PIX_TILE)  # 254
NP_PAD = NTILES_FULL * PIX_TILE     # 2080768

_BINSIZE = np.float32(np.float64(1.000001) / (LUT_DIM - 1))
_SCALE = np.float32(1.0) / _BINSIZE

_PROG_CACHE = {}


def _build_consts(lut):
    lut = np.asarray(lut, dtype=np.float32)
    # corner windows: V[c, ib, ig, ir, db, dg, dr]
    V = np.lib.stride_tricks.sliding_window_view(lut, (2, 2, 2), axis=(1, 2, 3))
    # cell linear index = ib*1024 + ig*32 + ir ; corner j = dr + 2*dg + 4*db
    V = V.reshape(3, NCELL, 2, 2, 2)            # (c, cell, db, dg, dr)
    V = np.transpose(V, (0, 1, 2, 3, 4)).reshape(3, NCELL, 8)  # j = db*4+dg*2+dr
    # reorder j so that j = bit0->dr? We want row j bits: dr=bit0, dg=bit1, db=bit2
    # current flatten gives index db*4+dg*2+dr which IS bit2=db,bit1=dg,bit0=dr. OK.
    # sign trick: weights are built as prod of p_c in {frac, frac-1}; flip sign
    # of table entries where the number of zero-bits of j is odd.
    j = np.arange(8)
    pc = np.array([bin(x).count("1") for x in j])
    sign = np.where((3 - pc) % 2 == 1, -1.0, 1.0).astype(np.float32)
    Vs = V * sign[None, None, :]

    tab = np.zeros((16, NCELL, 2), dtype=np.float16)
    tab[0:8, :, 0] = np.transpose(Vs[0], (1, 0))   # c0, corner j at row j, dpos0
    tab[0:8, :, 1] = np.transpose(Vs[1], (1, 0))   # c1 at dpos1
    tab[8:16, :, 0] = np.transpose(Vs[2], (1, 0))  # c2 at rows 8+j, dpos0
    tab128 = np.tile(tab, (8, 1, 1))               # replicate for 8 groups

    # identity for PE transpose (64x64)
    idn = np.eye(64, dtype=np.float32)

    # replication matmul weights (compact rhs rows r=c*8+g, ones at row 24):
    # psum[16g+jj] = a_{csrc}[g] + (bit(jj%8) ? +0.5 : -0.5)
    rep = np.zeros((3, 32, 128), dtype=np.float32)
    for i, (csrc, bit) in enumerate([(0, 0), (1, 1), (2, 2)]):  # PR, PG, PB
        for g in range(8):
            for jj in range(16):
                m = 16 * g + jj
                b = ((jj % 8) >> bit) & 1  # slot kappa(j) = j % 8
                rep[i, csrc * 8 + g, m] = 1.0
                rep[i, 24, m] = 0.5 if b else -0.5

    # slot-sum selection matrices [128, 24] fp16, m = c*8 + g
    selA = np.zeros((128, 24), dtype=np.float16)  # dpos 0: c0 (j<8), c2 (j>=8)
    selB = np.zeros((128, 24), dtype=np.float16)  # dpos 1: c1 (j<8)
    for g in range(8):
        for jj in range(16):
            p = 16 * g + jj
            if jj < 8:
                selA[p, 0 * 8 + g] = 1.0
                selB[p, 1 * 8 + g] = 1.0
            else:
                selA[p, 2 * 8 + g] = 1.0
    return tab128, idn, rep, selA, selB


def _build_program(ntiles, n_cores):
    import concourse.bacc as bacc
    import concourse.mybir as mybir
    from concourse.tile import TileContext

    fp32 = mybir.dt.float32
    fp16 = mybir.dt.float16
    i16 = mybir.dt.int16
    STT = mybir.AluOpType

    np_pix = ntiles * PIX_TILE
    nc = bacc.Bacc("TRN2", target_bir_lowering=False, debug=False,
                   num_devices=n_cores)
    d_x = nc.dram_tensor("x", [3, np_pix], fp32, kind="ExternalInput")
    d_tab = nc.dram_tensor("tab", [128, NCELL, 2], fp16, kind="ExternalInput")
    d_idn = nc.dram_tensor("idn", [64, 64], fp32, kind="ExternalInput")
    d_rep = nc.dram_tensor("rep", [3, 32, 128], fp32, kind="ExternalInput")
    d_selA = nc.dram_tensor("selA", [128, 24], fp16, kind="ExternalInput")
    d_selB = nc.dram_tensor("selB", [128, 24], fp16, kind="ExternalInput")
    d_out = nc.dram_tensor("out", [3, np_pix], fp32, kind="ExternalOutput")

    with TileContext(nc) as tc:
        with (
            tc.tile_pool(name="const", bufs=1) as cpool,
            tc.tile_pool(name="work", bufs=2) as wpool,
            tc.tile_pool(name="big", bufs=2) as bpool,
            tc.tile_pool(name="scr", bufs=1) as spool,
            tc.tile_pool(name="ps", bufs=1, space="PSUM") as pspool,
            tc.tile_pool(name="ps2", bufs=1, space="PSUM") as ps2pool,
        ):
            t_tab = cpool.tile([128, NCELL, 2], fp16)
            t_idn = cpool.tile([64, 64], fp32)
            t_rep = cpool.tile([32, 3, 128], fp32)
            t_selA = cpool.tile([128, 24], fp16)
            t_selB = cpool.tile([128, 24], fp16)
            nc.sync.dma_start(t_tab[:, :, :], d_tab.ap()[:, :, :])
            nc.sync.dma_start(t_idn[:, :], d_idn.ap()[:, :])
            for i in range(3):
                nc.sync.dma_start(t_rep[:, i, :], d_rep.ap()[i, :, :])
            nc.sync.dma_start(t_selA[:, :], d_selA.ap()[:, :])
            nc.sync.dma_start(t_selB[:, :], d_selB.ap()[:, :])

            for ti in range(ntiles):
                base = ti * PIX_TILE
                # ---------- pixel-parallel path: cell indices ----------
                t_xpp = wpool.tile([64, 3, 8, 16], fp32, tag="xpp")
                for c in range(3):
                    src = d_x.ap()[c, base:base + PIX_TILE].rearrange(
                        "(g p u) -> p g u", g=8, p=64, u=16)
                    nc.sync.dma_start(t_xpp[:, c, :, :], src)
                t_xbpp = wpool.tile([64, 3, 8, 16], fp32, tag="xbpp")
                nc.scalar.activation(t_xbpp[:, :, :, :], t_xpp[:, :, :, :],
                                     mybir.ActivationFunctionType.Copy,
                                     bias=-0.5, scale=float(_SCALE))
                t_fipp = wpool.tile([64, 3, 8, 16], i16, tag="fipp")
                nc.vector.tensor_copy(t_fipp[:, :, :, :], t_xbpp[:, :, :, :])
                t_ffpp = wpool.tile([64, 3, 8, 16], fp32, tag="ffpp")
                nc.vector.tensor_copy(t_ffpp[:, :, :, :], t_fipp[:, :, :, :])
                t_cell = wpool.tile([64, 8, 16], fp32, tag="cell")
                # cell = (ff_b*32 + ff_g)*32 + ff_r
                nc.vector.scalar_tensor_tensor(
                    t_cell[:, :, :], t_ffpp[:, 2, :, :], 32.0,
                    t_ffpp[:, 1, :, :], STT.mult, STT.add)
                nc.vector.scalar_tensor_tensor(
                    t_cell[:, :, :], t_cell[:, :, :], 32.0,
                    t_ffpp[:, 0, :, :], STT.mult, STT.add)
                # one transpose [64,128] -> psum [128,64]: row 16g+j = wrapped idx
                p_T = pspool.tile([128, 64], fp32, tag="pT")
                nc.tensor.transpose(
                    p_T[:, :],
                    t_cell[:, :, :].rearrange('p a b -> p (a b)'),
                    t_idn[:, :])
                t_idx = wpool.tile([128, 64], i16, tag="idx")
                nc.vector.tensor_copy(t_idx[:, :], p_T[:, :])

                # ---------- gather ----------
                t_V = bpool.tile([128, N, 2], fp16, tag="V")
                nc.gpsimd.ap_gather(t_V[:, :, :], t_tab[:, :, :], t_idx[:, :],
                                    channels=128, num_elems=NCELL, d=2,
                                    num_idxs=N)

                # ---------- stream path: fractions (compact [25,N] layout) ----------
                t_X = bpool.tile([32, N], fp32, tag="X")
                nc.vector.memset(t_X[:, :], 1.0)
                for c in range(3):
                    src = d_x.ap()[c, base:base + PIX_TILE].rearrange(
                        "(g n) -> g n", g=8, n=N)
                    nc.sync.dma_start(t_X[c * 8:(c + 1) * 8, :], src)
                t_XB = spool.tile([32, N], fp32, tag="XB")
                nc.scalar.activation(t_XB[0:24, :], t_X[0:24, :],
                                     mybir.ActivationFunctionType.Copy,
                                     bias=-0.5, scale=float(_SCALE))
                t_FI = spool.tile([32, N], i16, tag="FI")
                t_FF = spool.tile([32, N], fp32, tag="FF")
                nc.vector.tensor_copy(t_FI[0:24, :], t_XB[0:24, :])
                nc.vector.tensor_copy(t_FF[0:24, :], t_FI[0:24, :])
                # a = (frac - 0.5) back into X rows 0:24
                nc.vector.tensor_sub(t_X[0:24, :], t_XB[0:24, :], t_FF[0:24, :])
                # replication matmuls (512-col chunks) + weights + products
                t_U = spool.tile([128, N], fp32, tag="U")
                t_W = spool.tile([128, N, 2], fp16, tag="W")
                t_P = bpool.tile([128, N, 2], fp16, tag="P")
                t_OUT = bpool.tile([24, N], fp32, tag="OUT")
                for ch in range(N // 512):
                    sl = slice(ch * 512, (ch + 1) * 512)
                    p_P = [ps2pool.tile([128, 512], fp32, tag=f"pP{i}",
                                        name=f"pP{i}_{ti}_{ch}")
                           for i in range(3)]
                    for i in range(3):
                        nc.tensor.matmul(p_P[i][:, :], t_rep[0:32, i, :],
                                         t_X[:, sl], start=True, stop=True)
                    # U = PG*PB ; W = U*PR (both dpos)
                    t_PG = spool.tile([128, N], fp32, tag="PG")
                    nc.vector.tensor_copy(t_PG[:, sl], p_P[1][:, :])
                    nc.vector.tensor_mul(t_U[:, sl], t_PG[:, sl], p_P[2][:, :])
                    nc.vector.tensor_mul(t_W[:, sl, 0], t_U[:, sl], p_P[0][:, :])
                    nc.vector.tensor_copy(t_W[:, sl, 1], t_W[:, sl, 0])
                    # P = V * W
                    nc.vector.tensor_mul(t_P[:, sl, :], t_V[:, sl, :],
                                         t_W[:, sl, :])
                    # slot-sum
                    p_S = ps2pool.tile([24, 512], fp32, tag="pS")
                    nc.tensor.matmul(p_S[:, :], t_selA[:, :], t_P[:, sl, 0],
                                     start=True, stop=False)
                    nc.tensor.matmul(p_S[:, :], t_selB[:, :], t_P[:, sl, 1],
                                     start=False, stop=True)
                    nc.vector.tensor_copy(t_OUT[:, sl], p_S[:, :])
                # ---------- store ----------
                for c in range(3):
                    dst = d_out.ap()[c, base:base + PIX_TILE].rearrange(
                        "(g n) -> g n", g=8, n=N)
                    nc.sync.dma_start(dst, t_OUT[c * 8:(c + 1) * 8, :])

    nc.compile()
    return nc


def _get_program(ntiles, n_cores):
    key = (ntiles, n_cores)
    if key not in _PROG_CACHE:
        _PROG_CACHE[key] = _build_program(ntiles, n_cores)
    return _PROG_CACHE[key]


def kernel(lut, x):
    from concourse import bass_utils

    lut = np.asarray(lut, dtype=np.float32)
    x = np.asarray(x, dtype=np.float32)
    B = x.shape[0]
    tab128, idn, rep, selA, selB = _build_consts(lut)

    nc = _get_program(NTILES_FULL, B)
    in_maps = []
    for b in range(B):
        xb = x[b].reshape(3, -1)
        xpad = np.zeros((3, NP_PAD), dtype=np.float32)
        xpad[:, :xb.shape[1]] = xb
        in_maps.append({
            "x": xpad, "tab": tab128, "idn": idn, "rep": rep,
            "selA": selA, "selB": selB,
        })
    res = bass_utils.run_bass_kernel_spmd(nc, in_maps, core_ids=list(range(B)))
    outs = []
    for b in range(B):
        o = res.results[b]["out"][:, :NPIX]
        outs.append(o.reshape(3, 1080, 1920))
    return np.stack(outs).astype(np.float32)


# revision 22
# speedup vs baseline: 9.6500x; 9.6500x over previous
"""3D LUT trilinear interpolation (color grading) on 8 Trainium2 NeuronCores.

Strategy (per core, data-parallel over batch):
  - Pack the 33^3 LUT into per-cell corner tables (32^3 cells x 8 corners x 3
    channels), fp16, laid out so one GPSIMD ap_gather index per pixel fetches
    all 24 values across 16 partition rows (d=2).
  - Per 8192-pixel tile: compute cell indices pixel-parallel, PE-transpose
    them into the gather's wrapped [16 x 64] index layout; compute fractions
    in stream layout, PE 0/1-matmuls replicate them into all 16 slot rows
    with +-0.5 bias folded in; 2 DVE ops build the 16 signed trilinear
    weights; multiply with gathered corners; 2 PE matmuls per 512-chunk
    contract the 8 corners per channel into PSUM.
"""

import numpy as np

LUT_DIM = 33
NCELL = 32 * 32 * 32      # 32768
N = 1024                  # stream pixels per 16-partition group per tile
PIX_TILE = 8 * N          # 8192 pixels per tile
NPIX = 3 * 1080 * 1920 // 3  # per-core pixels = 2073600
NTILES_FULL = -(-NPIX // PIX_TILE)  # 254
NP_PAD = NTILES_FULL * PIX_TILE     # 2080768

_BINSIZE = np.float32(np.float64(1.000001) / (LUT_DIM - 1))
_SCALE = np.float32(1.0) / _BINSIZE

_PROG_CACHE = {}


def _build_consts(lut):
    lut = np.asarray(lut, dtype=np.float32)
    # corner windows: V[c, ib, ig, ir, db, dg, dr]
    V = np.lib.stride_tricks.sliding_window_view(lut, (2, 2, 2), axis=(1, 2, 3))
    # cell linear index = ib*1024 + ig*32 + ir ; corner j = dr + 2*dg + 4*db
    V = V.reshape(3, NCELL, 2, 2, 2)            # (c, cell, db, dg, dr)
    V = np.transpose(V, (0, 1, 2, 3, 4)).reshape(3, NCELL, 8)  # j = db*4+dg*2+dr
    # reorder j so that j = bit0->dr? We want row j bits: dr=bit0, dg=bit1, db=bit2
    # current flatten gives index db*4+dg*2+dr which IS bit2=db,bit1=dg,bit0=dr. OK.
    # sign trick: weights are built as prod of p_c in {frac, frac-1}; flip sign
    # of table entries where the number of zero-bits of j is odd.
    j = np.arange(8)
    pc = np.array([bin(x).count("1") for x in j])
    sign = np.where((3 - pc) % 2 == 1, -1.0, 1.0).astype(np.float32)
    Vs = V * sign[None, None, :]

    tab = np.zeros((16, NCELL, 2), dtype=np.float16)
    tab[0:8, :, 0] = np.transpose(Vs[0], (1, 0))   # c0, corner j at row j, dpos0
    tab[0:8, :, 1] = np.transpose(Vs[1], (1, 0))   # c1 at dpos1
    tab[8:16, :, 0] = np.transpose(Vs[2], (1, 0))  # c2 at rows 8+j, dpos0
    tab128 = np.tile(tab, (8, 1, 1))               # replicate for 8 groups

    # identity for PE transpose (64x64)
    idn = np.eye(64, dtype=np.float32)

    # replication matmul weights (compact rhs rows r=c*8+g, ones at row 24):
    # psum[16g+jj] = a_{csrc}[g] + (bit(jj%8) ? +0.5 : -0.5)
    rep = np.zeros((3, 32, 128), dtype=np.float32)
    for i, (csrc, bit) in enumerate([(0, 0), (1, 1), (2, 2)]):  # PR, PG, PB
        for g in range(8):
            for jj in range(16):
                m = 16 * g + jj
                b = ((jj % 8) >> bit) & 1  # slot kappa(j) = j % 8
                rep[i, csrc * 8 + g, m] = 1.0
                rep[i, 24, m] = 0.5 if b else -0.5

    # slot-sum selection matrices [128, 24] fp16, m = c*8 + g
    selA = np.zeros((128, 24), dtype=np.float16)  # dpos 0: c0 (j<8), c2 (j>=8)
    selB = np.zeros((128, 24), dtype=np.float16)  # dpos 1: c1 (j<8)
    for g in range(8):
        for jj in range(16):
            p = 16 * g + jj
            if jj < 8:
                selA[p, 0 * 8 + g] = 1.0
                selB[p, 1 * 8 + g] = 1.0
            else:
                selA[p, 2 * 8 + g] = 1.0
    return tab128, idn, rep, selA, selB


def _build_program(ntiles, n_cores, skip_gather=False, minimal=False, reps=1):
    import concourse.bacc as bacc
    import concourse.mybir as mybir
    from concourse.tile import TileContext

    fp32 = mybir.dt.float32
    fp16 = mybir.dt.float16
    i16 = mybir.dt.int16
    STT = mybir.AluOpType

    np_pix = ntiles * PIX_TILE
    nc = bacc.Bacc("TRN2", target_bir_lowering=False, debug=False,
                   num_devices=n_cores)
    d_x = nc.dram_tensor("x", [3, np_pix], fp32, kind="ExternalInput")
    d_tab = nc.dram_tensor("tab", [128, NCELL, 2], fp16, kind="ExternalInput")
    d_idn = nc.dram_tensor("idn", [64, 64], fp32, kind="ExternalInput")
    d_rep = nc.dram_tensor("rep", [3, 32, 128], fp32, kind="ExternalInput")
    d_selA = nc.dram_tensor("selA", [128, 24], fp16, kind="ExternalInput")
    d_selB = nc.dram_tensor("selB", [128, 24], fp16, kind="ExternalInput")
    d_out = nc.dram_tensor("out", [3, np_pix], fp32, kind="ExternalOutput")

    with TileContext(nc) as tc:
        with (
            tc.tile_pool(name="const", bufs=1) as cpool,
            tc.tile_pool(name="work", bufs=2) as wpool,
            tc.tile_pool(name="big", bufs=2) as bpool,
            tc.tile_pool(name="scr", bufs=1) as spool,
            tc.tile_pool(name="ps", bufs=2, space="PSUM") as pspool,
            tc.tile_pool(name="ps2", bufs=1, space="PSUM") as ps2pool,
        ):
            t_tab = cpool.tile([128, NCELL, 2], fp16)
            t_idn = cpool.tile([64, 64], fp32)
            t_rep = cpool.tile([32, 3, 128], fp32)
            t_selA = cpool.tile([128, 24], fp16)
            t_selB = cpool.tile([128, 24], fp16)
            nc.sync.dma_start(t_tab[:, :, :], d_tab.ap()[:, :, :])
            nc.sync.dma_start(t_idn[:, :], d_idn.ap()[:, :])
            for i in range(3):
                nc.sync.dma_start(t_rep[:, i, :], d_rep.ap()[i, :, :])
            nc.sync.dma_start(t_selA[:, :], d_selA.ap()[:, :])
            nc.sync.dma_start(t_selB[:, :], d_selB.ap()[:, :])

            for ti0 in range(ntiles * reps):
                ti = ti0 % ntiles
                base = ti * PIX_TILE
                # ---------- pixel-parallel path: cell indices ----------
                t_xpp = wpool.tile([64, 3, 8, 16], fp32, tag="xpp")
                for c in range(3):
                    src = d_x.ap()[c, base:base + PIX_TILE].rearrange(
                        "(g p u) -> p g u", g=8, p=64, u=16)
                    nc.gpsimd.dma_start(t_xpp[:, c, :, :], src)
                if minimal:
                    t_OUTm = bpool.tile([24, N], fp32, tag="OUT")
                    nc.vector.memset(t_OUTm[:, :], 0.0)
                    for c in range(3):
                        dst = d_out.ap()[c, base:base + PIX_TILE].rearrange(
                            "(g n) -> g n", g=8, n=N)
                        nc.sync.dma_start(dst, t_OUTm[c * 8:(c + 1) * 8, :])
                    continue
                t_xbpp = wpool.tile([64, 3, 8, 16], fp32, tag="xbpp")
                nc.scalar.activation(t_xbpp[:, :, :, :], t_xpp[:, :, :, :],
                                     mybir.ActivationFunctionType.Copy,
                                     bias=-0.5, scale=float(_SCALE))
                t_fipp = wpool.tile([64, 3, 8, 16], i16, tag="fipp")
                nc.vector.tensor_copy(t_fipp[:, :, :, :], t_xbpp[:, :, :, :])
                t_ffpp = wpool.tile([64, 3, 8, 16], fp32, tag="ffpp")
                nc.vector.tensor_copy(t_ffpp[:, :, :, :], t_fipp[:, :, :, :])
                t_cell = wpool.tile([64, 8, 16], fp32, tag="cell")
                # cell = (ff_b*32 + ff_g)*32 + ff_r
                nc.vector.scalar_tensor_tensor(
                    t_cell[:, :, :], t_ffpp[:, 2, :, :], 32.0,
                    t_ffpp[:, 1, :, :], STT.mult, STT.add)
                nc.vector.scalar_tensor_tensor(
                    t_cell[:, :, :], t_cell[:, :, :], 32.0,
                    t_ffpp[:, 0, :, :], STT.mult, STT.add)
                # one transpose [64,128] -> psum [128,64]: row 16g+j = wrapped idx
                p_T = pspool.tile([128, 64], fp32, tag="pT")
                nc.tensor.transpose(
                    p_T[:, :],
                    t_cell[:, :, :].rearrange('p a b -> p (a b)'),
                    t_idn[:, :])
                t_idx = wpool.tile([128, 64], i16, tag="idx")
                nc.vector.tensor_copy(t_idx[:, :], p_T[:, :])

                # ---------- gather ----------
                t_V = bpool.tile([128, N, 2], fp16, tag="V")
                if skip_gather:
                    nc.vector.memset(t_V[:, :, :], 0.25)
                if not skip_gather:
                    nc.gpsimd.ap_gather(t_V[:, :, :], t_tab[:, :, :],
                                        t_idx[:, :], channels=128,
                                        num_elems=NCELL, d=2, num_idxs=N)

                # ---------- stream path: fractions (compact [25,N] layout) ----------
                t_X = bpool.tile([32, N], fp32, tag="X")
                nc.vector.memset(t_X[:, :], 1.0)
                for c in range(3):
                    src = d_x.ap()[c, base:base + PIX_TILE].rearrange(
                        "(g n) -> g n", g=8, n=N)
                    nc.sync.dma_start(t_X[c * 8:(c + 1) * 8, :], src)
                t_XB = wpool.tile([32, N], fp32, tag="XB")
                nc.scalar.activation(t_XB[0:24, :], t_X[0:24, :],
                                     mybir.ActivationFunctionType.Copy,
                                     bias=-0.5, scale=float(_SCALE))
                t_FI = wpool.tile([32, N], i16, tag="FI")
                t_FF = wpool.tile([32, N], fp32, tag="FF")
                nc.vector.tensor_copy(t_FI[0:24, :], t_XB[0:24, :])
                nc.vector.tensor_copy(t_FF[0:24, :], t_FI[0:24, :])
                # a = (frac - 0.5) back into X rows 0:24
                nc.vector.tensor_sub(t_X[0:24, :], t_XB[0:24, :], t_FF[0:24, :])
                # replication matmuls (512-col chunks) + weights + products
                t_U = spool.tile([128, N], fp32, tag="U")
                t_W = spool.tile([128, N, 2], fp16, tag="W")
                t_P = bpool.tile([128, N, 2], fp16, tag="P")
                t_OUT = bpool.tile([24, N], fp32, tag="OUT")
                for ch in range(N // 512):
                    sl = slice(ch * 512, (ch + 1) * 512)
                    p_P = [ps2pool.tile([128, 512], fp32, tag=f"pP{i}",
                                        name=f"pP{i}_{ti}_{ch}")
                           for i in range(3)]
                    for i in range(3):
                        nc.tensor.matmul(p_P[i][:, :], t_rep[0:32, i, :],
                                         t_X[:, sl], start=True, stop=True)
                    # U = PG*PB ; W = U*PR (both dpos)
                    t_PG = spool.tile([128, N], fp32, tag="PG")
                    nc.vector.tensor_copy(t_PG[:, sl], p_P[1][:, :])
                    nc.vector.tensor_mul(t_U[:, sl], t_PG[:, sl], p_P[2][:, :])
                    nc.vector.tensor_mul(t_W[:, sl, 0], t_U[:, sl], p_P[0][:, :])
                    nc.vector.tensor_copy(t_W[:, sl, 1], t_W[:, sl, 0])
                    # P = V * W
                    nc.vector.tensor_mul(t_P[:, sl, :], t_V[:, sl, :],
                                         t_W[:, sl, :])
                    # slot-sum
                    p_S = ps2pool.tile([24, 512], fp32, tag="pS")
                    nc.tensor.matmul(p_S[:, :], t_selA[:, :], t_P[:, sl, 0],
                                     start=True, stop=False)
                    nc.tensor.matmul(p_S[:, :], t_selB[:, :], t_P[:, sl, 1],
                                     start=False, stop=True)
                    nc.vector.tensor_copy(t_OUT[:, sl], p_S[:, :])
                # ---------- store ----------
                for c in range(3):
                    dst = d_out.ap()[c, base:base + PIX_TILE].rearrange(
                        "(g n) -> g n", g=8, n=N)
                    nc.sync.dma_start(dst, t_OUT[c * 8:(c + 1) * 8, :])

    nc.compile()
    return nc


def _get_program(ntiles, n_cores):
    key = (ntiles, n_cores)
    if key not in _PROG_CACHE:
        _PROG_CACHE[key] = _build_program(ntiles, n_cores)
    return _PROG_CACHE[key]


def kernel(lut, x):
    from concourse import bass_utils

    lut = np.asarray(lut, dtype=np.float32)
    x = np.asarray(x, dtype=np.float32)
    B = x.shape[0]
    tab128, idn, rep, selA, selB = _build_consts(lut)

    nc = _get_program(NTILES_FULL, B)
    in_maps = []
    for b in range(B):
        xb = x[b].reshape(3, -1)
        xpad = np.zeros((3, NP_PAD), dtype=np.float32)
        xpad[:, :xb.shape[1]] = xb
        in_maps.append({
            "x": xpad, "tab": tab128, "idn": idn, "rep": rep,
            "selA": selA, "selB": selB,
        })
    res = bass_utils.run_bass_kernel_spmd(nc, in_maps, core_ids=list(range(B)))
    outs = []
    for b in range(B):
        o = res.results[b]["out"][:, :NPIX]
        outs.append(o.reshape(3, 1080, 1920))
    return np.stack(outs).astype(np.float32)


# revision 26
# speedup vs baseline: 20.0105x; 2.0736x over previous
"""3D LUT trilinear interpolation (color grading) on 8 Trainium2 NeuronCores.

Strategy (per core, data-parallel over batch):
  - Pack the 33^3 LUT into per-cell corner tables (32^3 cells x 8 corners x 3
    channels), fp16, laid out so one GPSIMD ap_gather index per pixel fetches
    all 24 values across 16 partition rows (d=2).
  - Per 8192-pixel tile: compute cell indices pixel-parallel, PE-transpose
    them into the gather's wrapped [16 x 64] index layout; compute fractions
    in stream layout, PE 0/1-matmuls replicate them into all 16 slot rows
    with +-0.5 bias folded in; 2 DVE ops build the 16 signed trilinear
    weights; multiply with gathered corners; 2 PE matmuls per 512-chunk
    contract the 8 corners per channel into PSUM.
"""

import numpy as np

LUT_DIM = 33
NCELL = 32 * 32 * 32      # 32768
N = 1024                  # stream pixels per 16-partition group per tile
PIX_TILE = 8 * N          # 8192 pixels per tile
NPIX = 3 * 1080 * 1920 // 3  # per-core pixels = 2073600
NTILES_FULL = -(-NPIX // PIX_TILE)  # 254
NP_PAD = NTILES_FULL * PIX_TILE     # 2080768

_BINSIZE = np.float32(np.float64(1.000001) / (LUT_DIM - 1))
_SCALE = np.float32(1.0) / _BINSIZE

_PROG_CACHE = {}


def _build_consts(lut):
    lut = np.asarray(lut, dtype=np.float32)
    # corner windows: V[c, ib, ig, ir, db, dg, dr]
    V = np.lib.stride_tricks.sliding_window_view(lut, (2, 2, 2), axis=(1, 2, 3))
    # cell linear index = ib*1024 + ig*32 + ir ; corner j = dr + 2*dg + 4*db
    V = V.reshape(3, NCELL, 2, 2, 2)            # (c, cell, db, dg, dr)
    V = np.transpose(V, (0, 1, 2, 3, 4)).reshape(3, NCELL, 8)  # j = db*4+dg*2+dr
    # reorder j so that j = bit0->dr? We want row j bits: dr=bit0, dg=bit1, db=bit2
    # current flatten gives index db*4+dg*2+dr which IS bit2=db,bit1=dg,bit0=dr. OK.
    # sign trick: weights are built as prod of p_c in {frac, frac-1}; flip sign
    # of table entries where the number of zero-bits of j is odd.
    j = np.arange(8)
    pc = np.array([bin(x).count("1") for x in j])
    sign = np.where((3 - pc) % 2 == 1, -1.0, 1.0).astype(np.float32)
    Vs = V * sign[None, None, :]

    tab = np.zeros((16, NCELL, 2), dtype=np.float16)
    tab[0:8, :, 0] = np.transpose(Vs[0], (1, 0))   # c0, corner j at row j, dpos0
    tab[0:8, :, 1] = np.transpose(Vs[1], (1, 0))   # c1 at dpos1
    tab[8:16, :, 0] = np.transpose(Vs[2], (1, 0))  # c2 at rows 8+j, dpos0
    tab128 = np.tile(tab, (8, 1, 1))               # replicate for 8 groups

    # identity for PE transpose (64x64)
    idn = np.eye(64, dtype=np.float32)

    # replication matmul weights (compact rhs rows r=c*8+g, ones at row 24):
    # psum[16g+jj] = a_{csrc}[g] + (bit(jj%8) ? +0.5 : -0.5)
    rep = np.zeros((3, 32, 128), dtype=np.float32)
    for i, (csrc, bit) in enumerate([(0, 0), (1, 1), (2, 2)]):  # PR, PG, PB
        for g in range(8):
            for jj in range(16):
                m = 16 * g + jj
                b = ((jj % 8) >> bit) & 1  # slot kappa(j) = j % 8
                rep[i, csrc * 8 + g, m] = 1.0
                rep[i, 24, m] = 0.5 if b else -0.5

    # slot-sum selection matrices [128, 24] fp16, m = c*8 + g
    selA = np.zeros((128, 24), dtype=np.float16)  # dpos 0: c0 (j<8), c2 (j>=8)
    selB = np.zeros((128, 24), dtype=np.float16)  # dpos 1: c1 (j<8)
    for g in range(8):
        for jj in range(16):
            p = 16 * g + jj
            if jj < 8:
                selA[p, 0 * 8 + g] = 1.0
                selB[p, 1 * 8 + g] = 1.0
            else:
                selA[p, 2 * 8 + g] = 1.0
    return tab128, idn, rep, selA, selB


def _build_program(ntiles, n_cores, skip_gather=False, minimal=False, reps=1):
    import concourse.bacc as bacc
    import concourse.mybir as mybir
    from concourse.tile import TileContext

    fp32 = mybir.dt.float32
    fp16 = mybir.dt.float16
    i16 = mybir.dt.int16
    STT = mybir.AluOpType

    np_pix = ntiles * PIX_TILE
    nc = bacc.Bacc("TRN2", target_bir_lowering=False, debug=False,
                   num_devices=n_cores)
    d_x = nc.dram_tensor("x", [3, np_pix], fp32, kind="ExternalInput")
    d_tab = nc.dram_tensor("tab", [128, NCELL, 2], fp16, kind="ExternalInput")
    d_idn = nc.dram_tensor("idn", [64, 64], fp32, kind="ExternalInput")
    d_rep = nc.dram_tensor("rep", [3, 32, 128], fp32, kind="ExternalInput")
    d_selA = nc.dram_tensor("selA", [128, 24], fp16, kind="ExternalInput")
    d_selB = nc.dram_tensor("selB", [128, 24], fp16, kind="ExternalInput")
    d_out = nc.dram_tensor("out", [3, np_pix], fp32, kind="ExternalOutput")

    with TileContext(nc) as tc:
        with (
            tc.tile_pool(name="const", bufs=1) as cpool,
            tc.tile_pool(name="work", bufs=2) as wpool,
            tc.tile_pool(name="big", bufs=2) as bpool,
            tc.tile_pool(name="scr", bufs=1) as spool,
            tc.tile_pool(name="ps", bufs=2, space="PSUM") as pspool,
            tc.tile_pool(name="ps2", bufs=1, space="PSUM") as ps2pool,
        ):
            t_tab = cpool.tile([128, NCELL, 2], fp16)
            t_idn = cpool.tile([64, 64], fp32)
            t_rep = cpool.tile([32, 3, 128], fp32)
            t_selA = cpool.tile([128, 24], fp16)
            t_selB = cpool.tile([128, 24], fp16)
            nc.sync.dma_start(t_tab[:, :, :], d_tab.ap()[:, :, :])
            nc.sync.dma_start(t_idn[:, :], d_idn.ap()[:, :])
            for i in range(3):
                nc.sync.dma_start(t_rep[:, i, :], d_rep.ap()[i, :, :])
            nc.sync.dma_start(t_selA[:, :], d_selA.ap()[:, :])
            nc.sync.dma_start(t_selB[:, :], d_selB.ap()[:, :])

            for ti0 in range(ntiles * reps):
                ti = ti0 % ntiles
                base = ti * PIX_TILE
                # ---------- pixel-parallel path: cell indices ----------
                t_xpp = wpool.tile([64, 3, 8, 16], fp32, tag="xpp")
                for c in range(3):
                    srcp = d_x.ap()[c, base:base + PIX_TILE].rearrange(
                        "(g p u) -> p g u", g=8, p=64, u=16)
                    nc.gpsimd.dma_start(t_xpp[:, c, :, :], srcp)
                if minimal:
                    t_OUTm = bpool.tile([24, N], fp32, tag="OUT")
                    nc.vector.memset(t_OUTm[:, :], 0.0)
                    for c in range(3):
                        dst = d_out.ap()[c, base:base + PIX_TILE].rearrange(
                            "(g n) -> g n", g=8, n=N)
                        nc.sync.dma_start(dst, t_OUTm[c * 8:(c + 1) * 8, :])
                    continue
                t_xbpp = wpool.tile([64, 3, 8, 16], fp32, tag="xbpp")
                nc.scalar.activation(t_xbpp[:, :, :, :], t_xpp[:, :, :, :],
                                     mybir.ActivationFunctionType.Copy,
                                     bias=-0.5, scale=float(_SCALE))
                t_fipp = wpool.tile([64, 3, 8, 16], i16, tag="fipp")
                nc.vector.tensor_copy(t_fipp[:, :, :, :], t_xbpp[:, :, :, :])
                t_ffpp = wpool.tile([64, 3, 8, 16], fp32, tag="ffpp")
                nc.vector.tensor_copy(t_ffpp[:, :, :, :], t_fipp[:, :, :, :])
                t_cell = wpool.tile([64, 8, 16], fp32, tag="cell")
                # cell = (ff_b*32 + ff_g)*32 + ff_r
                nc.vector.scalar_tensor_tensor(
                    t_cell[:, :, :], t_ffpp[:, 2, :, :], 32.0,
                    t_ffpp[:, 1, :, :], STT.mult, STT.add)
                nc.vector.scalar_tensor_tensor(
                    t_cell[:, :, :], t_cell[:, :, :], 32.0,
                    t_ffpp[:, 0, :, :], STT.mult, STT.add)
                # one transpose [64,128] -> psum [128,64]: row 16g+j = wrapped idx
                p_T = pspool.tile([128, 64], fp32, tag="pT")
                nc.tensor.transpose(
                    p_T[:, :],
                    t_cell[:, :, :].rearrange('p a b -> p (a b)'),
                    t_idn[:, :])
                t_idx = wpool.tile([128, 64], i16, tag="idx")
                nc.vector.tensor_copy(t_idx[:, :], p_T[:, :])

                # ---------- gather ----------
                t_V = bpool.tile([128, N, 2], fp16, tag="V")
                if skip_gather:
                    nc.vector.memset(t_V[:, :, :], 0.25)
                if not skip_gather:
                    nc.gpsimd.ap_gather(t_V[:, :, :], t_tab[:, :, :],
                                        t_idx[:, :], channels=128,
                                        num_elems=NCELL, d=2, num_idxs=N)

                # ---------- stream path: fractions (compact [25,N] layout) ----------
                t_X = bpool.tile([32, N], fp32, tag="X")
                nc.vector.memset(t_X[:, :], 1.0)
                for c in range(3):
                    srcs = d_x.ap()[c, base:base + PIX_TILE].rearrange(
                        "(g n) -> g n", g=8, n=N)
                    nc.sync.dma_start(t_X[c * 8:(c + 1) * 8, :], srcs)
                t_XB = wpool.tile([32, N], fp32, tag="XB")
                nc.scalar.activation(t_XB[0:24, :], t_X[0:24, :],
                                     mybir.ActivationFunctionType.Copy,
                                     bias=-0.5, scale=float(_SCALE))
                t_FI = wpool.tile([32, N], i16, tag="FI")
                t_FF = wpool.tile([32, N], fp32, tag="FF")
                nc.vector.tensor_copy(t_FI[0:24, :], t_XB[0:24, :])
                nc.vector.tensor_copy(t_FF[0:24, :], t_FI[0:24, :])
                # a = (frac - 0.5) back into X rows 0:24
                nc.vector.tensor_sub(t_X[0:24, :], t_XB[0:24, :], t_FF[0:24, :])
                # replication matmuls (512-col chunks) + weights + products
                t_U = spool.tile([128, N], fp32, tag="U")
                t_W = spool.tile([128, N, 2], fp16, tag="W")
                t_P = bpool.tile([128, N, 2], fp16, tag="P")
                t_OUT = bpool.tile([24, N], fp32, tag="OUT")
                for ch in range(N // 512):
                    sl = slice(ch * 512, (ch + 1) * 512)
                    p_P = [ps2pool.tile([128, 512], fp32, tag=f"pP{i}",
                                        name=f"pP{i}_{ti}_{ch}")
                           for i in range(3)]
                    for i in range(3):
                        nc.tensor.matmul(p_P[i][:, :], t_rep[0:32, i, :],
                                         t_X[:, sl], start=True, stop=True)
                    # U = PG*PB ; W = U*PR (both dpos)
                    t_PG = spool.tile([128, N], fp32, tag="PG")
                    nc.vector.tensor_copy(t_PG[:, sl], p_P[1][:, :])
                    nc.vector.tensor_mul(t_U[:, sl], t_PG[:, sl], p_P[2][:, :])
                    nc.vector.tensor_mul(t_W[:, sl, 0], t_U[:, sl], p_P[0][:, :])
                    nc.vector.tensor_copy(t_W[:, sl, 1], t_W[:, sl, 0])
                    # P = V * W
                    nc.vector.tensor_mul(t_P[:, sl, :], t_V[:, sl, :],
                                         t_W[:, sl, :])
                    # slot-sum
                    p_S = ps2pool.tile([24, 512], fp32, tag="pS")
                    nc.tensor.matmul(p_S[:, :], t_selA[:, :], t_P[:, sl, 0],
                                     start=True, stop=False)
                    nc.tensor.matmul(p_S[:, :], t_selB[:, :], t_P[:, sl, 1],
                                     start=False, stop=True)
                    nc.vector.tensor_copy(t_OUT[:, sl], p_S[:, :])
                # ---------- store ----------
                for c in range(3):
                    dst = d_out.ap()[c, base:base + PIX_TILE].rearrange(
                        "(g n) -> g n", g=8, n=N)
                    nc.sync.dma_start(dst, t_OUT[c * 8:(c + 1) * 8, :])

    nc.compile()
    return nc


def _get_program(ntiles, n_cores):
    key = (ntiles, n_cores)
    if key not in _PROG_CACHE:
        _PROG_CACHE[key] = _build_program(ntiles, n_cores)
    return _PROG_CACHE[key]


def kernel(lut, x):
    from concourse import bass_utils

    lut = np.asarray(lut, dtype=np.float32)
    x = np.asarray(x, dtype=np.float32)
    B = x.shape[0]
    tab128, idn, rep, selA, selB = _build_consts(lut)

    nc = _get_program(NTILES_FULL, B)
    in_maps = []
    for b in range(B):
        xb = x[b].reshape(3, -1)
        xpad = np.zeros((3, NP_PAD), dtype=np.float32)
        xpad[:, :xb.shape[1]] = xb
        in_maps.append({
            "x": xpad, "tab": tab128, "idn": idn, "rep": rep,
            "selA": selA, "selB": selB,
        })
    res = bass_utils.run_bass_kernel_spmd(nc, in_maps, core_ids=list(range(B)))
    outs = []
    for b in range(B):
        o = res.results[b]["out"][:, :NPIX]
        outs.append(o.reshape(3, 1080, 1920))
    return np.stack(outs).astype(np.float32)
